# revision 31
# baseline (speedup 1.0000x reference)
"""MHA kernel for trn2, 8 NeuronCores, head-sharded (2 heads/core).

Per core c (heads 2c, 2c+1):
  qT/kT = (w_{q,k} shard).T @ x.T  -> [128, T] bf16 (rows 0:64 head a, 64:128 head b)
  v     = x @ w_v shard            -> [T, 128]
  per q-tile (512) x key-block (128):
     S^T = kT_blk.T @ qT   (row-tiled pair, K=64 per head, one [128,1024] psum)
     eS  = exp(S^T)        (ACT; q pre-scaled by 1/8 on host)
     y'[65,512] += Vp[kb].T @ eS   (Vp = [V | ones]; row 64 = Z = sum exp)
  y0s/y1s = bf16 casts of y'[0:64]; Z rows -> DRAM
  out-proj per head (K=64): outT{0,1}[:, qtile] = wo[head rows].T @ y{0,1}s
  (out-proj of q-tile i is emitted inside q-tile i+1's loop to hide latency)

Host: normalizes per-head partials by Z and sums over cores/heads.
"""

import numpy as np
import ml_dtypes

import concourse.bacc as bacc
import concourse.mybir as mybir
from concourse.tile import TileContext
from concourse.bass_utils import run_bass_kernel_spmd
from concourse.masks import make_identity

BF16 = ml_dtypes.bfloat16
F32 = mybir.dt.float32
BF = mybir.dt.bfloat16
EXP = mybir.ActivationFunctionType.Exp

B, T, C = 1, 4096, 1024
H, D = 16, 64
NCORES = 8
P = 128
CB = C // P          # 8 contraction blocks
KB = T // P          # 32 key blocks
QTS = T // 512       # 8 q tiles

_cached = None


def build_bass():
    global _cached
    if _cached is not None:
        return _cached

    nc = bacc.Bacc("TRN2", target_bir_lowering=False, name="mha_head_sharded")

    xT = nc.dram_tensor("xT", (C, T), BF, kind="ExternalInput")
    wq = nc.dram_tensor("wq", (C, P), BF, kind="ExternalInput")
    wk = nc.dram_tensor("wk", (C, P), BF, kind="ExternalInput")
    wv = nc.dram_tensor("wv", (C, P), BF, kind="ExternalInput")
    wo = nc.dram_tensor("wo", (P, C), BF, kind="ExternalInput")
    outT0 = nc.dram_tensor("outT0", (C, T), BF, kind="ExternalOutput")
    outT1 = nc.dram_tensor("outT1", (C, T), BF, kind="ExternalOutput")
    Zt = nc.dram_tensor("Zt", (2, T), F32, kind="ExternalOutput")

    with TileContext(nc) as tc:
        with (
            tc.tile_pool(name="const", bufs=1) as const,
            tc.tile_pool(name="work", bufs=3) as work,
            tc.tile_pool(name="psS", bufs=2, space="PSUM") as psS,
            tc.tile_pool(name="psY", bufs=1, space="PSUM") as psY,
            tc.tile_pool(name="psO", bufs=2, space="PSUM") as psO,
        ):
            # ---- load inputs (weights on gpsimd queue, x on sync queue) ----
            wqs = const.tile([P, CB, P], BF)
            nc.gpsimd.dma_start(wqs[:], wq[:, :].rearrange("(cb p) f -> p cb f", p=P))
            wks = const.tile([P, CB, P], BF)
            nc.gpsimd.dma_start(wks[:], wk[:, :].rearrange("(cb p) f -> p cb f", p=P))
            wvs = const.tile([P, CB, P], BF)
            nc.gpsimd.dma_start(wvs[:], wv[:, :].rearrange("(cb p) f -> p cb f", p=P))
            wos = const.tile([P, C], BF)
            nc.gpsimd.dma_start(wos[:], wo[:, :])
            xTs = const.tile([P, CB, T], BF)
            xT_r = xT[:, :].rearrange("(cb p) t -> p cb t", p=P)
            for tt in range(QTS):
                nc.sync.dma_start(xTs[:, :, tt * 512:(tt + 1) * 512],
                                  xT_r[:, :, tt * 512:(tt + 1) * 512])

            # ---- kT (all tok tiles) + qT (first tile only; rest deferred) ----
            qTs = const.tile([P, T], BF)
            kTs = const.tile([P, T], BF)
            for tt in range(QTS):
                pq = psS.tile([P, 1024], F32, tag="s")
                for cb in range(CB):
                    nc.tensor.matmul(
                        pq[:, 0:512], wks[:, cb, :], xTs[:, cb, tt * 512:(tt + 1) * 512],
                        start=(cb == 0), stop=(cb == CB - 1))
                if tt == 0:
                    for cb in range(CB):
                        nc.tensor.matmul(
                            pq[:, 512:1024], wqs[:, cb, :], xTs[:, cb, 0:512],
                            start=(cb == 0), stop=(cb == CB - 1))
                nc.scalar.copy(kTs[:, tt * 512:(tt + 1) * 512], pq[:, 0:512])
                if tt == 0:
                    nc.vector.tensor_copy(qTs[:, 0:512], pq[:, 512:1024])

            def emit_qt_proj(tokt):
                # compute qT for tok tile `tokt` using a shared psum slot
                pqd = psO.tile([P, 512], F32, tag="po", name="pqd")
                for cb in range(CB):
                    nc.tensor.matmul(
                        pqd[:, :], wqs[:, cb, :], xTs[:, cb, tokt * 512:(tokt + 1) * 512],
                        start=(cb == 0), stop=(cb == CB - 1))
                nc.vector.tensor_copy(qTs[:, tokt * 512:(tokt + 1) * 512], pqd[:])

            # ---- V: compute v^T with stationary weights, then PE-transpose ----
            ident = const.tile([P, P], BF)
            make_identity(nc, ident[:])
            Vp0 = const.tile([P, KB, 65], BF)
            Vp1 = const.tile([P, KB, 65], BF)
            nc.vector.memset(Vp0[:, :, 64:65], 1.0)
            nc.vector.memset(Vp1[:, :, 64:65], 1.0)
            for tt in range(QTS):
                pvt = psO.tile([P, 512], F32, tag="po", name="pvt")
                for cb in range(CB):
                    nc.tensor.matmul(
                        pvt[:, :], wvs[:, cb, :], xTs[:, cb, tt * 512:(tt + 1) * 512],
                        start=(cb == 0), stop=(cb == CB - 1))
                vts = work.tile([P, 512], BF, tag="vts")
                nc.vector.tensor_copy(vts[:], pvt[:])
                for j in range(4):
                    tb = tt * 4 + j
                    ptp = psO.tile([P, P], BF, tag="po", name="ptp")
                    nc.tensor.transpose(ptp[:], vts[:, j * P:(j + 1) * P], ident[:])
                    nc.scalar.copy(Vp0[:, tb, 0:64], ptp[:, 0:64])
                    nc.vector.tensor_copy(Vp1[:, tb, 0:64], ptp[:, 64:128])

            # ---- main loop: attention with deferred out-projection ----
            def emit_outproj(dep, fb, flush=False):
                # row-tiled concurrent pair: head a on array rows 0:64, head b on 64:128
                yns, q0 = dep
                if flush and fb % 2 == 1:
                    st = psS.tile([P, 1024], F32, tag="s", name="st")
                    poA, poB = st[:, 0:512], st[:, 512:1024]
                else:
                    poA = psO.tile([P, 512], F32, tag="po", name="poA")
                    poB = psO.tile([P, 512], F32, tag="po", name="poB")
                nc.tensor.matmul(poA[:, :], wos[0:64, fb * P:(fb + 1) * P],
                                 yns[0:64, :], start=True, stop=True)
                nc.tensor.matmul(poB[:, :], wos[64:128, fb * P:(fb + 1) * P],
                                 yns[64:128, :], start=True, stop=True)
                ocA = work.tile([P, 512], BF, tag="oc")
                nc.vector.tensor_copy(ocA[:], poA[:])
                nc.gpsimd.dma_start(outT0[fb * P:(fb + 1) * P, q0:q0 + 512], ocA[:])
                ocB = work.tile([P, 512], BF, tag="oc")
                nc.vector.tensor_copy(ocB[:], poB[:])
                nc.sync.dma_start(outT1[fb * P:(fb + 1) * P, q0:q0 + 512], ocB[:])

            pending = None
            for qt in range(QTS):
                q0 = qt * 512
                y0 = psY.tile([65, 512], F32, tag="y0")
                y1 = psY.tile([65, 512], F32, tag="y1")
                for kb in range(KB):
                    k0 = kb * P
                    s = psS.tile([P, 1024], F32, tag="s")
                    nc.tensor.matmul(s[:, 0:512], kTs[0:64, k0:k0 + P],
                                     qTs[0:64, q0:q0 + 512], start=True, stop=True)
                    nc.tensor.matmul(s[:, 512:1024], kTs[64:128, k0:k0 + P],
                                     qTs[64:128, q0:q0 + 512], start=True, stop=True)
                    eS = work.tile([P, 1024], BF, tag="es")
                    nc.scalar.activation(eS[:], s[:], EXP)
                    nc.tensor.matmul(y0[:, :], Vp0[:, kb, :], eS[:, 0:512],
                                     start=(kb == 0), stop=(kb == KB - 1))
                    nc.tensor.matmul(y1[:, :], Vp1[:, kb, :], eS[:, 512:1024],
                                     start=(kb == 0), stop=(kb == KB - 1))
                    if pending is not None and kb % 4 == 1:
                        emit_outproj(pending, kb // 4)
                    if kb == 14 and qt + 1 < QTS:
                        emit_qt_proj(qt + 1)

                # casts release Y psum; Z rows go straight to DRAM.
                # yns rows 64:128 (head b) arrive via SBUF->SBUF DMA partition move.
                yns = work.tile([P, 512], BF, tag="yns", bufs=2)
                y1t = work.tile([64, 512], BF, tag="y1t")
                nc.vector.tensor_copy(yns[0:64, :], y0[0:64, :])
                nc.vector.tensor_copy(y1t[:], y1[0:64, :])
                nc.gpsimd.dma_start(yns[64:128, :], y1t[:])
                ztile = work.tile([65, 1024], F32, tag="zt")
                nc.vector.tensor_copy(ztile[64:65, 0:512], y0[64:65, :])
                nc.vector.tensor_copy(ztile[64:65, 512:1024], y1[64:65, :])
                nc.gpsimd.dma_start(Zt[0:1, q0:q0 + 512], ztile[64:65, 0:512])
                nc.gpsimd.dma_start(Zt[1:2, q0:q0 + 512], ztile[64:65, 512:1024])
                pending = (yns, q0)

            for fb in range(CB):
                emit_outproj(pending, fb, flush=True)

    nc.compile()
    _cached = nc
    return nc


def make_in_maps(x, w_qkv, w_out):
    """x [1,T,C] f32, w_qkv [C, 3C] f32, w_out [C, C] f32 -> per-core input dicts."""
    scale = 1.0 / np.sqrt(np.float32(D))
    xT = np.ascontiguousarray(x.reshape(T, C).T).astype(BF16)  # [C, T]
    in_maps = []
    for c in range(NCORES):
        cols = slice(P * c, P * (c + 1))
        wq = np.ascontiguousarray(w_qkv[:, 0:C][:, cols] * scale).astype(BF16)
        wk = np.ascontiguousarray(w_qkv[:, C:2 * C][:, cols]).astype(BF16)
        wv = np.ascontiguousarray(w_qkv[:, 2 * C:3 * C][:, cols]).astype(BF16)
        wo = np.ascontiguousarray(w_out[P * c:P * (c + 1), :]).astype(BF16)
        in_maps.append({"xT": xT, "wq": wq, "wk": wk, "wv": wv, "wo": wo})
    return in_maps


def run(x, w_qkv, w_out, trace=False):
    nc = build_bass()
    in_maps = make_in_maps(x, w_qkv, w_out)
    res = run_bass_kernel_spmd(nc, in_maps, core_ids=list(range(NCORES)), trace=trace)
    acc = np.zeros((C, T), dtype=np.float32)
    for r in res.results:
        z = r["Zt"]  # [2, T]
        acc += r["outT0"].astype(np.float32) / z[0][None, :]
        acc += r["outT1"].astype(np.float32) / z[1][None, :]
    out = np.ascontiguousarray(acc.T).reshape(B, T, C)
    return out, res


def kernel(x, w_qkv, w_out):
    out, _ = run(x, w_qkv, w_out, trace=False)
    return out


# revision 32
# speedup vs baseline: 1.0170x; 1.0170x over previous
"""MHA kernel for trn2, 8 NeuronCores, head-sharded (2 heads/core).

Per core c (heads 2c, 2c+1):
  qT/kT = (w_{q,k} shard).T @ x.T  -> [128, T] bf16 (rows 0:64 head a, 64:128 head b)
  v     = x @ w_v shard            -> [T, 128]
  per q-tile (512) x key-block (128):
     S^T = kT_blk.T @ qT   (row-tiled pair, K=64 per head, one [128,1024] psum)
     eS  = exp(S^T)        (ACT; q pre-scaled by 1/8 on host)
     y'[65,512] += Vp[kb].T @ eS   (Vp = [V | ones]; row 64 = Z = sum exp)
  y0s/y1s = bf16 casts of y'[0:64]; Z rows -> DRAM
  out-proj per head (K=64): outT{0,1}[:, qtile] = wo[head rows].T @ y{0,1}s
  (out-proj of q-tile i is emitted inside q-tile i+1's loop to hide latency)

Host: normalizes per-head partials by Z and sums over cores/heads.
"""

import numpy as np
import ml_dtypes

import concourse.bacc as bacc
import concourse.mybir as mybir
from concourse.tile import TileContext
from concourse.bass_utils import run_bass_kernel_spmd
from concourse.masks import make_identity

BF16 = ml_dtypes.bfloat16
F32 = mybir.dt.float32
BF = mybir.dt.bfloat16
EXP = mybir.ActivationFunctionType.Exp

B, T, C = 1, 4096, 1024
H, D = 16, 64
NCORES = 8
P = 128
CB = C // P          # 8 contraction blocks
KB = T // P          # 32 key blocks
QTS = T // 512       # 8 q tiles

_cached = None


def build_bass():
    global _cached
    if _cached is not None:
        return _cached

    nc = bacc.Bacc("TRN2", target_bir_lowering=False, name="mha_head_sharded")

    xT = nc.dram_tensor("xT", (C, T), BF, kind="ExternalInput")
    wq = nc.dram_tensor("wq", (C, P), BF, kind="ExternalInput")
    wk = nc.dram_tensor("wk", (C, P), BF, kind="ExternalInput")
    wv = nc.dram_tensor("wv", (C, P), BF, kind="ExternalInput")
    wo = nc.dram_tensor("wo", (P, C), BF, kind="ExternalInput")
    outT0 = nc.dram_tensor("outT0", (C, T), BF, kind="ExternalOutput")
    outT1 = nc.dram_tensor("outT1", (C, T), BF, kind="ExternalOutput")
    Zt = nc.dram_tensor("Zt", (2, T), F32, kind="ExternalOutput")

    with TileContext(nc) as tc:
        with (
            tc.tile_pool(name="const", bufs=1) as const,
            tc.tile_pool(name="work", bufs=3) as work,
            tc.tile_pool(name="psS", bufs=2, space="PSUM") as psS,
            tc.tile_pool(name="psY", bufs=1, space="PSUM") as psY,
            tc.tile_pool(name="psO", bufs=2, space="PSUM") as psO,
        ):
            # ---- load inputs (weights on gpsimd queue, x on sync queue) ----
            wqs = const.tile([P, CB, P], BF)
            nc.gpsimd.dma_start(wqs[:], wq[:, :].rearrange("(cb p) f -> p cb f", p=P))
            wks = const.tile([P, CB, P], BF)
            nc.gpsimd.dma_start(wks[:], wk[:, :].rearrange("(cb p) f -> p cb f", p=P))
            wvs = const.tile([P, CB, P], BF)
            nc.gpsimd.dma_start(wvs[:], wv[:, :].rearrange("(cb p) f -> p cb f", p=P))
            wos = const.tile([P, C], BF)
            nc.gpsimd.dma_start(wos[:], wo[:, :])
            xTs = const.tile([P, CB, T], BF)
            xT_r = xT[:, :].rearrange("(cb p) t -> p cb t", p=P)
            for tt in range(QTS):
                nc.sync.dma_start(xTs[:, :, tt * 512:(tt + 1) * 512],
                                  xT_r[:, :, tt * 512:(tt + 1) * 512])

            # ---- kT (all tok tiles) + qT (first tile only; rest deferred) ----
            qTs = const.tile([P, T], BF)
            kTs = const.tile([P, T], BF)
            for tt in range(QTS):
                pq = psS.tile([P, 1024], F32, tag="s")
                for cb in range(CB):
                    nc.tensor.matmul(
                        pq[:, 0:512], wks[:, cb, :], xTs[:, cb, tt * 512:(tt + 1) * 512],
                        start=(cb == 0), stop=(cb == CB - 1))
                if tt == 0:
                    for cb in range(CB):
                        nc.tensor.matmul(
                            pq[:, 512:1024], wqs[:, cb, :], xTs[:, cb, 0:512],
                            start=(cb == 0), stop=(cb == CB - 1))
                nc.scalar.copy(kTs[:, tt * 512:(tt + 1) * 512], pq[:, 0:512])
                if tt == 0:
                    nc.vector.tensor_copy(qTs[:, 0:512], pq[:, 512:1024])

            def emit_qt_proj(tokt):
                # compute qT for tok tile `tokt` using a shared psum slot
                pqd = psO.tile([P, 512], F32, tag="po", name="pqd")
                for cb in range(CB):
                    nc.tensor.matmul(
                        pqd[:, :], wqs[:, cb, :], xTs[:, cb, tokt * 512:(tokt + 1) * 512],
                        start=(cb == 0), stop=(cb == CB - 1))
                nc.vector.tensor_copy(qTs[:, tokt * 512:(tokt + 1) * 512], pqd[:])

            # ---- V (natural layout via x^T-stationary matmuls, + ones col) ----
            Vp0 = const.tile([P, KB, 65], BF)
            Vp1 = const.tile([P, KB, 65], BF)
            nc.vector.memset(Vp0[:, :, 64:65], 1.0)
            nc.vector.memset(Vp1[:, :, 64:65], 1.0)
            for tb in range(KB):
                pv = psO.tile([P, 512], F32, tag="po", name="pv")
                for cb in range(CB):
                    nc.tensor.matmul(
                        pv[:, 0:P], xTs[:, cb, tb * P:(tb + 1) * P], wvs[:, cb, :],
                        start=(cb == 0), stop=(cb == CB - 1))
                nc.scalar.copy(Vp0[:, tb, 0:64], pv[:, 0:64])
                nc.vector.tensor_copy(Vp1[:, tb, 0:64], pv[:, 64:128])

            # ---- main loop: attention with deferred out-projection ----
            def emit_outproj(dep, fb, flush=False):
                # row-tiled concurrent pair: head a on array rows 0:64, head b on 64:128
                yns, q0 = dep
                if flush and fb % 2 == 1:
                    st = psS.tile([P, 1024], F32, tag="s", name="st")
                    poA, poB = st[:, 0:512], st[:, 512:1024]
                else:
                    poA = psO.tile([P, 512], F32, tag="po", name="poA")
                    poB = psO.tile([P, 512], F32, tag="po", name="poB")
                nc.tensor.matmul(poA[:, :], wos[0:64, fb * P:(fb + 1) * P],
                                 yns[0:64, :], start=True, stop=True)
                nc.tensor.matmul(poB[:, :], wos[64:128, fb * P:(fb + 1) * P],
                                 yns[64:128, :], start=True, stop=True)
                ocA = work.tile([P, 512], BF, tag="oc")
                nc.vector.tensor_copy(ocA[:], poA[:])
                nc.gpsimd.dma_start(outT0[fb * P:(fb + 1) * P, q0:q0 + 512], ocA[:])
                ocB = work.tile([P, 512], BF, tag="oc")
                nc.vector.tensor_copy(ocB[:], poB[:])
                nc.sync.dma_start(outT1[fb * P:(fb + 1) * P, q0:q0 + 512], ocB[:])

            pending = None
            for qt in range(QTS):
                q0 = qt * 512
                y0 = psY.tile([65, 512], F32, tag="y0")
                y1 = psY.tile([65, 512], F32, tag="y1")
                for kb in range(KB):
                    k0 = kb * P
                    s = psS.tile([P, 1024], F32, tag="s")
                    nc.tensor.matmul(s[:, 0:512], kTs[0:64, k0:k0 + P],
                                     qTs[0:64, q0:q0 + 512], start=True, stop=True)
                    nc.tensor.matmul(s[:, 512:1024], kTs[64:128, k0:k0 + P],
                                     qTs[64:128, q0:q0 + 512], start=True, stop=True)
                    eS = work.tile([P, 1024], BF, tag="es")
                    nc.scalar.activation(eS[:], s[:], EXP)
                    nc.tensor.matmul(y0[:, :], Vp0[:, kb, :], eS[:, 0:512],
                                     start=(kb == 0), stop=(kb == KB - 1))
                    nc.tensor.matmul(y1[:, :], Vp1[:, kb, :], eS[:, 512:1024],
                                     start=(kb == 0), stop=(kb == KB - 1))
                    if pending is not None and kb % 4 == 1:
                        emit_outproj(pending, kb // 4)
                    if kb == 14 and qt + 1 < QTS:
                        emit_qt_proj(qt + 1)

                # casts release Y psum; Z rows go straight to DRAM.
                # yns rows 64:128 (head b) arrive via SBUF->SBUF DMA partition move.
                yns = work.tile([P, 512], BF, tag="yns", bufs=2)
                y1t = work.tile([64, 512], BF, tag="y1t")
                nc.vector.tensor_copy(yns[0:64, :], y0[0:64, :])
                nc.vector.tensor_copy(y1t[:], y1[0:64, :])
                nc.gpsimd.dma_start(yns[64:128, :], y1t[:])
                ztile = work.tile([65, 1024], F32, tag="zt")
                nc.vector.tensor_copy(ztile[64:65, 0:512], y0[64:65, :])
                nc.vector.tensor_copy(ztile[64:65, 512:1024], y1[64:65, :])
                nc.gpsimd.dma_start(Zt[0:1, q0:q0 + 512], ztile[64:65, 0:512])
                nc.gpsimd.dma_start(Zt[1:2, q0:q0 + 512], ztile[64:65, 512:1024])
                pending = (yns, q0)

            for fb in range(CB):
                emit_outproj(pending, fb, flush=True)

    nc.compile()
    _cached = nc
    return nc


def make_in_maps(x, w_qkv, w_out):
    """x [1,T,C] f32, w_qkv [C, 3C] f32, w_out [C, C] f32 -> per-core input dicts."""
    scale = 1.0 / np.sqrt(np.float32(D))
    xT = np.ascontiguousarray(x.reshape(T, C).T).astype(BF16)  # [C, T]
    in_maps = []
    for c in range(NCORES):
        cols = slice(P * c, P * (c + 1))
        wq = np.ascontiguousarray(w_qkv[:, 0:C][:, cols] * scale).astype(BF16)
        wk = np.ascontiguousarray(w_qkv[:, C:2 * C][:, cols]).astype(BF16)
        wv = np.ascontiguousarray(w_qkv[:, 2 * C:3 * C][:, cols]).astype(BF16)
        wo = np.ascontiguousarray(w_out[P * c:P * (c + 1), :]).astype(BF16)
        in_maps.append({"xT": xT, "wq": wq, "wk": wk, "wv": wv, "wo": wo})
    return in_maps


def run(x, w_qkv, w_out, trace=False):
    nc = build_bass()
    in_maps = make_in_maps(x, w_qkv, w_out)
    res = run_bass_kernel_spmd(nc, in_maps, core_ids=list(range(NCORES)), trace=trace)
    acc = np.zeros((C, T), dtype=np.float32)
    for r in res.results:
        z = r["Zt"]  # [2, T]
        acc += r["outT0"].astype(np.float32) / z[0][None, :]
        acc += r["outT1"].astype(np.float32) / z[1][None, :]
    out = np.ascontiguousarray(acc.T).reshape(B, T, C)
    return out, res


def kernel(x, w_qkv, w_out):
    out, _ = run(x, w_qkv, w_out, trace=False)
    return out
